# revision 20
# baseline (speedup 1.0000x reference)
"""AnomalyAttention Trainium2 kernel (8 NeuronCores, SPMD data-parallel over batch).

Math (per b,h):
  series = softmax(causal_mask(Q K^T / 8))          = E / sum(E)
  prior  = rownorm(exp(-(l-s)^2 / (2 sigma'^2)))    = G / sum(G)   (banded: |l-s|<=16 matters)
  fused  = g*series + (1-g)*prior ; renormalize     (sum == 1 -> renorm skipped, err ~1e-6)
  out    = fused @ V = a*(E@V) + b*(G@V),  a = g/sum(E), b = (1-g)/sum(G)  per row.

v3 structure:
  - scores computed TRANSPOSED (S^T = K Q^T, [s,l] layout) so exp(S^T) in SBUF is
    directly the lhsT of the PV matmul: no PE transposes / PSUM round trip for E.
  - row sums obtained via a ones-column appended to V (U_ext = A^T @ [V|1]):
    sum lands in column 64 of the PSUM result; no ACT accumulator reads.
  - Gaussian prior in [l,s] band layout (160 window per 128-chunk), premultiplied
    input (m * d2 on gpsimd), ONE merged exp per pair, PE-transposed to [s,l].
  - bf16 matmuls; normalization applied after PV on [128,64] tiles.
"""

import math
from contextlib import ExitStack

import ml_dtypes
import numpy as np

import concourse.bass as bass
import concourse.mybir as mybir
import concourse.tile as tile
from concourse import bacc
from concourse.bass_utils import run_bass_kernel_spmd

F32 = mybir.dt.float32
BF16 = mybir.dt.bfloat16
AF = mybir.ActivationFunctionType
OP = mybir.AluOpType

B, L, H, E = 16, 512, 8, 64
NCORES = 8
BPC = B // NCORES  # batches per core
PC = 128           # partition chunk
NCH = L // PC      # 4 chunks of 128 rows
BAND = 160         # gaussian band window (s in [128*li-16, 128*li+144))
BOFF = 16
EXT = 65           # V columns + ones column
MASKVAL = -240.0   # exp(0.125*(x-240)) <= e^-24 ~ 0
LN3 = math.log(3.0)

_CACHE = {}
LAST_RESULT = None


def _consts():
    ident = np.eye(PC, dtype=ml_dtypes.bfloat16)
    # mask for S^T diag block: -240 where l < s  (strict lower triangle: col j < row i)
    mtri_t = np.tril(np.full((PC, PC), MASKVAL, dtype=np.float32), k=-1).astype(
        ml_dtypes.bfloat16
    )
    # dist2 variants [3, 128, BAND]: d2[p, j] = (j - 16 - p)^2 ; poisoned out-of-range
    p = np.arange(PC)[:, None]
    j = np.arange(BAND)[None, :]
    d2 = ((j - BOFF - p) ** 2).astype(np.float32)
    d2_first = d2.copy()
    d2_first[:, :BOFF] = 1e30  # li=0: s = j-16 < 0 invalid
    d2_last = d2.copy()
    d2_last[:, 144:] = 1e30    # li=3: s = 352+j >= 512 invalid (j >= 144+16)
    dist2 = np.stack([d2_first, d2, d2_last])
    ones = np.ones((1, PC), dtype=np.float32)
    return ident, mtri_t, dist2, ones


def _build():
    if "nc" in _CACHE:
        return _CACHE["nc"]
    nc = bacc.Bacc()
    ident_np, mtri_np, dist2_np, ones_np = _consts()

    q_h = nc.dram_tensor("queries", [BPC, L, H, E], F32, kind="ExternalInput")
    k_h = nc.dram_tensor("keys", [BPC, L, H, E], F32, kind="ExternalInput")
    v_h = nc.dram_tensor("values", [BPC, L, H, E], F32, kind="ExternalInput")
    sig_h = nc.dram_tensor("sigma", [BPC, L, H], F32, kind="ExternalInput")
    hgl_h = nc.dram_tensor("hgl", [1, H], F32, kind="ExternalInput")
    out_h = nc.dram_tensor("out", [BPC, L, H, E], F32, kind="ExternalOutput")

    ident_d = nc.inline_tensor(ident_np, name="identc")
    mtri_d = nc.inline_tensor(mtri_np, name="mtric")
    dist2_d = nc.inline_tensor(dist2_np, name="dist2c")
    ones_d = nc.inline_tensor(ones_np, name="onesc")

    with ExitStack() as ctx:
        tc = ctx.enter_context(tile.TileContext(nc))
        const = ctx.enter_context(tc.tile_pool(name="const", bufs=1))
        qkT = ctx.enter_context(tc.tile_pool(name="qkT", bufs=2))
        vpool = ctx.enter_context(tc.tile_pool(name="vpool", bufs=2))
        spool = ctx.enter_context(tc.tile_pool(name="spool", bufs=2))
        etpool = ctx.enter_context(tc.tile_pool(name="etpool", bufs=3))
        gpool = ctx.enter_context(tc.tile_pool(name="gpool", bufs=3))
        gtpool = ctx.enter_context(tc.tile_pool(name="gtpool", bufs=2))
        small = ctx.enter_context(tc.tile_pool(name="small", bufs=3))
        outp = ctx.enter_context(tc.tile_pool(name="outp", bufs=2))
        tmpp = ctx.enter_context(tc.tile_pool(name="tmpp", bufs=3))
        ps_s = ctx.enter_context(tc.tile_pool(name="ps_s", bufs=2, space="PSUM"))
        ps_t = ctx.enter_context(tc.tile_pool(name="ps_t", bufs=2, space="PSUM"))
        ps_u = ctx.enter_context(tc.tile_pool(name="ps_u", bufs=2, space="PSUM"))
        dram = ctx.enter_context(tc.tile_pool(name="dram", bufs=2, space="DRAM"))

        # ---- constants ----
        ident = const.tile([PC, PC], BF16, tag="ident")
        nc.sync.dma_start(ident, ident_d[:, :])
        mtri = const.tile([PC, PC], BF16, tag="mtri")
        nc.sync.dma_start(mtri, mtri_d[:, :])
        d2sb = const.tile([PC, 3 * BAND], F32, tag="d2sb")
        for v in range(3):
            nc.sync.dma_start(d2sb[:, v * BAND:(v + 1) * BAND], dist2_d[v, :, :])
        ones_sb = const.tile([1, PC], F32, tag="ones")
        nc.sync.dma_start(ones_sb, ones_d[:, :])

        # ---- gates ----
        hgl_sb = const.tile([1, H], F32, tag="hgl")
        nc.sync.dma_start(hgl_sb, hgl_h[:, :])
        ge = const.tile([1, H], F32, tag="ge")
        nc.scalar.activation(ge, hgl_sb, AF.Exp, scale=-1.0)
        gp = const.tile([1, H], F32, tag="gp")
        nc.vector.tensor_scalar_add(gp, ge, 1.0)
        gate = const.tile([1, H], F32, tag="gate")
        nc.vector.reciprocal(gate, gp)  # sigmoid
        gb_ps = ps_s.tile([PC, L], F32, tag="S")
        nc.tensor.matmul(gb_ps[:, 0:H], ones_sb, gate, start=True, stop=True)
        gates_b = const.tile([PC, H], F32, tag="gatesb")
        nc.vector.tensor_copy(gates_b, gb_ps[:, 0:H])
        omg_b = const.tile([PC, H], F32, tag="omgb")
        nc.vector.tensor_scalar(omg_b, gates_b, -1.0, 1.0, OP.mult, OP.add)

        for bi in range(BPC):
            # ---- Q/K: SWDGE cast f32->bf16 straight to DRAM scratch ----
            qscr = dram.tile([L, H * E], BF16, tag="qscr")
            kscr = dram.tile([L, H * E], BF16, tag="kscr")
            nc.gpsimd.dma_start(qscr[:, :], q_h[bi, :, :, :])
            nc.gpsimd.dma_start(kscr[:, :], k_h[bi, :, :, :])
            QT = []  # two [128, 1024] tiles: halves to∈{0,1} and {2,3}
            KT = []
            for half in range(2):
                qt = qkT.tile([PC, 1024], BF16, tag=f"qT{half}")
                kt = qkT.tile([PC, 1024], BF16, tag=f"kT{half}")
                for j2 in range(2):
                    to = 2 * half + j2
                    nc.sync.dma_start_transpose(
                        qt[:, j2 * L:(j2 + 1) * L], qscr[:, to * PC:(to + 1) * PC]
                    )
                    nc.sync.dma_start_transpose(
                        kt[:, j2 * L:(j2 + 1) * L], kscr[:, to * PC:(to + 1) * PC]
                    )
                QT.append(qt)
                KT.append(kt)

            # ---- V with ones column appended per head: [128, 8*65] ----
            # Vn_ext: natural rows; Vs_ext: rows shifted by -16 (5 tiles)
            Vn = []
            for t in range(4):
                vn = vpool.tile([PC, H * EXT], BF16, tag=f"vn{t}")
                nc.gpsimd.dma_start(
                    vn[:, :].rearrange("p (h e) -> p h e", h=H)[:, :, 0:E],
                    v_h[bi, t * PC:(t + 1) * PC, :, :],
                )
                nc.gpsimd.memset(
                    vn[:, :].rearrange("p (h e) -> p h e", h=H)[:, :, E:EXT], 1.0
                )
                Vn.append(vn)
            Vs = []
            for t in range(5):
                vs = vpool.tile([PC, H * EXT], BF16, tag=f"vs{t}")
                vs3 = vs[:, :].rearrange("p (h e) -> p h e", h=H)
                if t in (0, 4):
                    # edge zero-pad rows first, then ones columns on top
                    nc.gpsimd.memset(vs[0:32, :], 0.0)
                nc.gpsimd.memset(vs3[:, :, E:EXT], 1.0)
                if t == 0:
                    nc.gpsimd.dma_start(
                        vs3[BOFF:PC, :, 0:E], v_h[bi, 0:PC - BOFF, :, :]
                    )
                elif t == 4:
                    nc.gpsimd.dma_start(
                        vs3[0:BOFF, :, 0:E], v_h[bi, L - BOFF:L, :, :]
                    )
                else:
                    nc.gpsimd.dma_start(
                        vs3[:, :, 0:E],
                        v_h[bi, t * PC - BOFF:(t + 1) * PC - BOFF, :, :],
                    )
                Vs.append(vs)

            # ---- sigma -> m = -0.5 / sigma'^2 ; [128, 32] col = 8*li + h ----
            sraw = spool.tile([PC, NCH * H], F32, tag="sraw")
            for c in range(NCH):
                nc.sync.dma_start(
                    sraw[:, c * H:(c + 1) * H], sig_h[bi, c * PC:(c + 1) * PC, :]
                )
            e5 = spool.tile([PC, NCH * H], F32, tag="e5")
            nc.scalar.activation(e5, sraw, AF.Exp, scale=-5.0)
            p1 = spool.tile([PC, NCH * H], F32, tag="p1")
            nc.vector.tensor_scalar_add(p1, e5, 1.0)
            sg = spool.tile([PC, NCH * H], F32, tag="sg")
            nc.vector.reciprocal(sg, p1)
            sg2 = spool.tile([PC, NCH * H], F32, tag="sg2")
            nc.vector.tensor_scalar_add(sg2, sg, 1e-5)
            p3 = spool.tile([PC, NCH * H], F32, tag="p3")
            nc.scalar.activation(p3, sg2, AF.Exp, scale=LN3)
            sm1 = spool.tile([PC, NCH * H], F32, tag="sm1")
            nc.vector.tensor_scalar_add(sm1, p3, -1.0)
            s2 = spool.tile([PC, NCH * H], F32, tag="s2")
            nc.vector.tensor_tensor(s2, sm1, sm1, OP.mult)
            r2 = spool.tile([PC, NCH * H], F32, tag="r2")
            nc.vector.reciprocal(r2, s2)
            mm = spool.tile([PC, NCH * H], F32, tag="mm")
            nc.vector.tensor_scalar_mul(mm, r2, -0.5)

            oslab = []
            for li in range(NCH):
                ot = outp.tile([PC, H * E], F32, tag=f"o{li}")
                oslab.append(ot)

            for h in range(H):
                th = h // 2
                po = 64 * (h % 2)
                vc = EXT * h

                # ---- E path in [s, l]: S^T = K Q^T, exp -> lhsT tiles ----
                ETs = []
                qko = (th % 2) * L
                for sj in range(NCH):
                    nw = L - sj * PC
                    S = ps_s.tile([PC, L], F32, tag="S")
                    lh = KT[th // 2][po:po + 64, qko + sj * PC:qko + (sj + 1) * PC]
                    rh = QT[th // 2][po:po + 64, qko + sj * PC:qko + L]
                    nc.tensor.matmul(S[:, 0:PC], lh, rh[:, 0:PC],
                                     start=True, stop=False)
                    nc.tensor.matmul(S[:, 0:PC], ident, mtri, start=False, stop=True)
                    if nw > PC:
                        nc.tensor.matmul(S[:, PC:nw], lh, rh[:, PC:nw],
                                         start=True, stop=True)
                    et = etpool.tile([PC, L], BF16, tag=f"ET{sj}")
                    nc.scalar.activation(et[:, 0:nw], S[:, 0:nw], AF.Exp, scale=0.125)
                    ETs.append(et)

                # ---- G path in [l, s]: premul + one exp + transpose ----
                gtmp = gpool.tile([PC, NCH * BAND], F32, tag="gtmp")
                for li in range(NCH):
                    var = 0 if li == 0 else (2 if li == 3 else 1)
                    nc.gpsimd.tensor_scalar_mul(
                        gtmp[:, li * BAND:(li + 1) * BAND],
                        d2sb[:, var * BAND:(var + 1) * BAND],
                        mm[:, li * H + h:li * H + h + 1],
                    )
                g4 = gpool.tile([PC, NCH * BAND], BF16, tag="g4")
                nc.scalar.activation(g4, gtmp, AF.Exp)
                gt_ps = ps_t.tile([PC, 1024], BF16, tag="Tps")
                for li in range(NCH):
                    nc.tensor.transpose(
                        gt_ps[:, li * PC:(li + 1) * PC],
                        g4[:, li * BAND:li * BAND + PC], ident,
                    )
                    nc.tensor.transpose(
                        gt_ps[0:32, L + li * PC:L + (li + 1) * PC],
                        g4[:, li * BAND + PC:(li + 1) * BAND], ident,
                    )
                gts = gtpool.tile([PC, 1024], BF16, tag="gts")
                nc.vector.tensor_copy(gts, gt_ps)

                # ---- PV matmuls with ones column: U_ext [128, 65] per chunk ----
                U1 = ps_u.tile([PC, L], F32, tag="U1")
                U2 = ps_u.tile([PC, L], F32, tag="U2")
                def et_slice(sj, li):
                    return ETs[sj][:, (li - sj) * PC:(li - sj + 1) * PC]

                for li in range(NCH):
                    for sj in range(li + 1):
                        nc.tensor.matmul(
                            U1[:, li * PC:li * PC + EXT],
                            et_slice(sj, li),
                            Vn[sj][:, vc:vc + EXT],
                            start=(sj == 0), stop=(sj == li),
                        )
                    nc.tensor.matmul(
                        U2[:, li * PC:li * PC + EXT],
                        gts[:, li * PC:(li + 1) * PC],
                        Vs[li][:, vc:vc + EXT],
                        start=True, stop=False,
                    )
                    nc.tensor.matmul(
                        U2[:, li * PC:li * PC + EXT],
                        gts[0:32, L + li * PC:L + (li + 1) * PC],
                        Vs[li + 1][0:32, vc:vc + EXT],
                        start=False, stop=True,
                    )

                # ---- normalization scalars from ones-column sums ----
                re = small.tile([PC, NCH], F32, tag="re")
                nc.vector.reciprocal(
                    re, U1[:, :].rearrange("p (c w) -> p c w", w=PC)[:, :, E]
                )
                rg = small.tile([PC, NCH], F32, tag="rg")
                nc.vector.reciprocal(
                    rg, U2[:, :].rearrange("p (c w) -> p c w", w=PC)[:, :, E]
                )
                av = small.tile([PC, NCH], F32, tag="av")
                nc.vector.tensor_scalar_mul(av, re, gates_b[:, h:h + 1])
                bv = small.tile([PC, NCH], F32, tag="bv")
                nc.vector.tensor_scalar_mul(bv, rg, omg_b[:, h:h + 1])

                for li in range(NCH):
                    t2 = tmpp.tile([PC, E], F32, tag="t2")
                    if bi == 1 and h >= 4:
                        nc.scalar.activation(
                            t2, U2[:, li * PC:li * PC + E], AF.Copy,
                            bias=0.0, scale=bv[:, li:li + 1],
                        )
                    else:
                        nc.vector.tensor_scalar_mul(
                            t2, U2[:, li * PC:li * PC + E], bv[:, li:li + 1]
                        )
                    nc.vector.scalar_tensor_tensor(
                        oslab[li][:, h * E:(h + 1) * E],
                        U1[:, li * PC:li * PC + E],
                        av[:, li:li + 1], t2, OP.mult, OP.add,
                    )

                if h == 3:
                    for li in range(NCH):
                        nc.sync.dma_start(
                            out_h[bi, li * PC:(li + 1) * PC, 0:4, :],
                            oslab[li][:, 0:4 * E],
                        )
            for li in range(NCH):
                nc.sync.dma_start(
                    out_h[bi, li * PC:(li + 1) * PC, 4:8, :], oslab[li][:, 4 * E:]
                )

    nc.compile()
    _CACHE["nc"] = nc
    return nc


def kernel(**inputs):
    global LAST_RESULT
    nc = _build()
    q = np.ascontiguousarray(inputs["queries"], dtype=np.float32)
    k = np.ascontiguousarray(inputs["keys"], dtype=np.float32)
    v = np.ascontiguousarray(inputs["values"], dtype=np.float32)
    sg = np.ascontiguousarray(inputs["sigma"], dtype=np.float32)
    hgl = np.ascontiguousarray(inputs["head_gate_logit"], dtype=np.float32).reshape(1, H)

    in_maps = []
    for c in range(NCORES):
        b0 = BPC * c
        in_maps.append({
            "queries": q[b0:b0 + BPC],
            "keys": k[b0:b0 + BPC],
            "values": v[b0:b0 + BPC],
            "sigma": sg[b0:b0 + BPC],
            "hgl": hgl,
        })
    res = run_bass_kernel_spmd(nc, in_maps, core_ids=list(range(NCORES)))
    LAST_RESULT = res
    out = np.concatenate([r["out"] for r in res.results], axis=0)
    return out.astype(np.float32)


# revision 31
# speedup vs baseline: 1.0481x; 1.0481x over previous
"""AnomalyAttention Trainium2 kernel (8 NeuronCores, SPMD data-parallel over batch).

Math (per b,h):
  series = softmax(causal_mask(Q K^T / 8))          = E / sum(E)
  prior  = rownorm(exp(-(l-s)^2 / (2 sigma'^2)))    = G / sum(G)   (banded: |l-s|<=16 matters)
  fused  = g*series + (1-g)*prior ; renormalize     (sum == 1 -> renorm skipped, err ~1e-6)
  out    = fused @ V = a*(E@V) + b*(G@V),  a = g/sum(E), b = (1-g)/sum(G)  per row.

v3 structure:
  - scores computed TRANSPOSED (S^T = K Q^T, [s,l] layout) so exp(S^T) in SBUF is
    directly the lhsT of the PV matmul: no PE transposes / PSUM round trip for E.
  - row sums obtained via a ones-column appended to V (U_ext = A^T @ [V|1]):
    sum lands in column 64 of the PSUM result; no ACT accumulator reads.
  - Gaussian prior in [l,s] band layout (160 window per 128-chunk), premultiplied
    input (m * d2 on gpsimd), ONE merged exp per pair, PE-transposed to [s,l].
  - bf16 matmuls; normalization applied after PV on [128,64] tiles.
"""

import math
from contextlib import ExitStack

import ml_dtypes
import numpy as np

import concourse.bass as bass
import concourse.mybir as mybir
import concourse.tile as tile
from concourse import bacc
from concourse.bass_utils import run_bass_kernel_spmd

F32 = mybir.dt.float32
BF16 = mybir.dt.bfloat16
AF = mybir.ActivationFunctionType
OP = mybir.AluOpType

B, L, H, E = 16, 512, 8, 64
NCORES = 8
BPC = B // NCORES  # batches per core
PC = 128           # partition chunk
NCH = L // PC      # 4 chunks of 128 rows
BAND = 160         # gaussian band window (s in [128*li-16, 128*li+144))
BOFF = 16
EXT = 65           # V columns + ones column
MASKVAL = -240.0   # exp(0.125*(x-240)) <= e^-24 ~ 0
LN3 = math.log(3.0)

_CACHE = {}
LAST_RESULT = None


def _consts():
    ident = np.eye(PC, dtype=ml_dtypes.bfloat16)
    # mask for S^T diag block: -240 where l < s  (strict lower triangle: col j < row i)
    mtri_t = np.tril(np.full((PC, PC), MASKVAL, dtype=np.float32), k=-1).astype(
        ml_dtypes.bfloat16
    )
    # dist2 variants [3, 128, BAND]: d2[p, j] = (j - 16 - p)^2 ; poisoned out-of-range
    p = np.arange(PC)[:, None]
    j = np.arange(BAND)[None, :]
    d2 = ((j - BOFF - p) ** 2).astype(np.float32)
    d2_first = d2.copy()
    d2_first[:, :BOFF] = 1e30  # li=0: s = j-16 < 0 invalid
    d2_last = d2.copy()
    d2_last[:, 144:] = 1e30    # li=3: s = 352+j >= 512 invalid (j >= 144+16)
    dist2 = np.stack([d2_first, d2, d2_last])
    ones = np.ones((1, PC), dtype=np.float32)
    return ident, mtri_t, dist2, ones


def _build():
    if "nc" in _CACHE:
        return _CACHE["nc"]
    nc = bacc.Bacc()
    ident_np, mtri_np, dist2_np, ones_np = _consts()

    q_h = nc.dram_tensor("queries", [BPC, L, H, E], F32, kind="ExternalInput")
    k_h = nc.dram_tensor("keys", [BPC, L, H, E], F32, kind="ExternalInput")
    v_h = nc.dram_tensor("values", [BPC, L, H, E], F32, kind="ExternalInput")
    sig_h = nc.dram_tensor("sigma", [BPC, L, H], F32, kind="ExternalInput")
    hgl_h = nc.dram_tensor("hgl", [1, H], F32, kind="ExternalInput")
    out_h = nc.dram_tensor("out", [BPC, L, H, E], F32, kind="ExternalOutput")

    ident_d = nc.inline_tensor(ident_np, name="identc")
    mtri_d = nc.inline_tensor(mtri_np, name="mtric")
    dist2_d = nc.inline_tensor(dist2_np, name="dist2c")
    ones_d = nc.inline_tensor(ones_np, name="onesc")

    with ExitStack() as ctx:
        tc = ctx.enter_context(tile.TileContext(nc))
        const = ctx.enter_context(tc.tile_pool(name="const", bufs=1))
        qkT = ctx.enter_context(tc.tile_pool(name="qkT", bufs=2))
        vpool = ctx.enter_context(tc.tile_pool(name="vpool", bufs=2))
        spool = ctx.enter_context(tc.tile_pool(name="spool", bufs=2))
        etpool = ctx.enter_context(tc.tile_pool(name="etpool", bufs=3))
        gpool = ctx.enter_context(tc.tile_pool(name="gpool", bufs=3))
        gtpool = ctx.enter_context(tc.tile_pool(name="gtpool", bufs=2))
        small = ctx.enter_context(tc.tile_pool(name="small", bufs=3))
        outp = ctx.enter_context(tc.tile_pool(name="outp", bufs=2))
        tmpp = ctx.enter_context(tc.tile_pool(name="tmpp", bufs=3))
        ps_s = ctx.enter_context(tc.tile_pool(name="ps_s", bufs=2, space="PSUM"))
        ps_t = ctx.enter_context(tc.tile_pool(name="ps_t", bufs=2, space="PSUM"))
        ps_u = ctx.enter_context(tc.tile_pool(name="ps_u", bufs=2, space="PSUM"))
        dram = ctx.enter_context(tc.tile_pool(name="dram", bufs=2, space="DRAM"))

        # ---- sigma -> m for both batches (hoisted: fast DMA, tiny ops) ----
        m_of = {}
        for bi in range(BPC):
                sraw = spool.tile([PC, NCH * H], F32, tag="sraw")
                for c in range(NCH):
                    nc.sync.dma_start(
                        sraw[:, c * H:(c + 1) * H], sig_h[bi, c * PC:(c + 1) * PC, :]
                    )
                e5 = spool.tile([PC, NCH * H], F32, tag="e5")
                nc.scalar.activation(e5, sraw, AF.Exp, scale=-5.0)
                p1 = spool.tile([PC, NCH * H], F32, tag="p1")
                nc.vector.tensor_scalar_add(p1, e5, 1.0)
                sg = spool.tile([PC, NCH * H], F32, tag="sg")
                nc.vector.reciprocal(sg, p1)
                sg2 = spool.tile([PC, NCH * H], F32, tag="sg2")
                nc.vector.tensor_scalar_add(sg2, sg, 1e-5)
                p3 = spool.tile([PC, NCH * H], F32, tag="p3")
                nc.scalar.activation(p3, sg2, AF.Exp, scale=LN3)
                sm1 = spool.tile([PC, NCH * H], F32, tag="sm1")
                nc.vector.tensor_scalar_add(sm1, p3, -1.0)
                s2 = spool.tile([PC, NCH * H], F32, tag="s2")
                nc.vector.tensor_tensor(s2, sm1, sm1, OP.mult)
                r2 = spool.tile([PC, NCH * H], F32, tag="r2")
                nc.vector.reciprocal(r2, s2)
                mm = spool.tile([PC, NCH * H], F32, tag="mm")
                nc.vector.tensor_scalar_mul(mm, r2, -0.5)

                m_of[bi] = mm

        # ---- constants ----
        ident = const.tile([PC, PC], BF16, tag="ident")
        nc.scalar.dma_start(ident, ident_d[:, :])
        mtri = const.tile([PC, PC], BF16, tag="mtri")
        nc.scalar.dma_start(mtri, mtri_d[:, :])
        d2sb = const.tile([PC, 3 * BAND], F32, tag="d2sb")
        for v in range(3):
            nc.scalar.dma_start(d2sb[:, v * BAND:(v + 1) * BAND], dist2_d[v, :, :])
        ones_sb = const.tile([1, PC], F32, tag="ones")
        nc.scalar.dma_start(ones_sb, ones_d[:, :])

        # ---- gates ----
        hgl_sb = const.tile([1, H], F32, tag="hgl")
        nc.scalar.dma_start(hgl_sb, hgl_h[:, :])
        ge = const.tile([1, H], F32, tag="ge")
        nc.scalar.activation(ge, hgl_sb, AF.Exp, scale=-1.0)
        gp = const.tile([1, H], F32, tag="gp")
        nc.vector.tensor_scalar_add(gp, ge, 1.0)
        gate = const.tile([1, H], F32, tag="gate")
        nc.vector.reciprocal(gate, gp)  # sigmoid
        gb_ps = ps_s.tile([PC, L], F32, tag="S")
        nc.tensor.matmul(gb_ps[:, 0:H], ones_sb, gate, start=True, stop=True)
        gates_b = const.tile([PC, H], F32, tag="gatesb")
        nc.vector.tensor_copy(gates_b, gb_ps[:, 0:H])
        omg_b = const.tile([PC, H], F32, tag="omgb")
        nc.vector.tensor_scalar(omg_b, gates_b, -1.0, 1.0, OP.mult, OP.add)

        for bi in range(BPC):
            # ---- Q/K: SWDGE cast f32->bf16 straight to DRAM scratch ----
            qscr = dram.tile([L, H * E], BF16, tag="qscr")
            kscr = dram.tile([L, H * E], BF16, tag="kscr")
            for to in range(4):
                nc.gpsimd.dma_start(
                    qscr[:, to * PC:(to + 1) * PC],
                    q_h[bi, :, 2 * to:2 * to + 2, :],
                )
                nc.gpsimd.dma_start(
                    kscr[:, to * PC:(to + 1) * PC],
                    k_h[bi, :, 2 * to:2 * to + 2, :],
                )
            QT = []  # four [128, 512] tiles, one per head pair
            KT = []
            for to in range(4):
                qt = qkT.tile([PC, L], BF16, tag=f"qT{to}")
                kt = qkT.tile([PC, L], BF16, tag=f"kT{to}")
                eng = nc.scalar if bi == 0 else nc.sync
                eng.dma_start_transpose(qt, qscr[:, to * PC:(to + 1) * PC])
                eng.dma_start_transpose(kt, kscr[:, to * PC:(to + 1) * PC])
                QT.append(qt)
                KT.append(kt)

            # ---- V with ones column appended per head: [128, 8*65] ----
            # Vn_ext: natural rows; Vs_ext: rows shifted by -16 (5 tiles)
            Vn = []
            for t in range(4):
                vn = vpool.tile([PC, H * EXT], BF16, tag=f"vn{t}")
                nc.gpsimd.dma_start(
                    vn[:, :].rearrange("p (h e) -> p h e", h=H)[:, :, 0:E],
                    v_h[bi, t * PC:(t + 1) * PC, :, :],
                )
                nc.gpsimd.memset(
                    vn[:, :].rearrange("p (h e) -> p h e", h=H)[:, :, E:EXT], 1.0
                )
                Vn.append(vn)
            Vs = []
            for t in range(5):
                vs = vpool.tile([PC, H * EXT], BF16, tag=f"vs{t}")
                vs3 = vs[:, :].rearrange("p (h e) -> p h e", h=H)
                if t in (0, 4):
                    # edge zero-pad rows first, then ones columns on top
                    nc.gpsimd.memset(vs[0:32, :], 0.0)
                nc.gpsimd.memset(vs3[:, :, E:EXT], 1.0)
                if t == 0:
                    nc.gpsimd.dma_start(
                        vs3[BOFF:PC, :, 0:E], v_h[bi, 0:PC - BOFF, :, :]
                    )
                elif t == 4:
                    nc.gpsimd.dma_start(
                        vs3[0:BOFF, :, 0:E], v_h[bi, L - BOFF:L, :, :]
                    )
                else:
                    nc.gpsimd.dma_start(
                        vs3[:, :, 0:E],
                        v_h[bi, t * PC - BOFF:(t + 1) * PC - BOFF, :, :],
                    )
                Vs.append(vs)

            if bi in m_of:
                mm = m_of[bi]
            else:
                sraw = spool.tile([PC, NCH * H], F32, tag="sraw")
                for c in range(NCH):
                    nc.sync.dma_start(
                        sraw[:, c * H:(c + 1) * H], sig_h[bi, c * PC:(c + 1) * PC, :]
                    )
                e5 = spool.tile([PC, NCH * H], F32, tag="e5")
                nc.scalar.activation(e5, sraw, AF.Exp, scale=-5.0)
                p1 = spool.tile([PC, NCH * H], F32, tag="p1")
                nc.vector.tensor_scalar_add(p1, e5, 1.0)
                sg = spool.tile([PC, NCH * H], F32, tag="sg")
                nc.vector.reciprocal(sg, p1)
                sg2 = spool.tile([PC, NCH * H], F32, tag="sg2")
                nc.vector.tensor_scalar_add(sg2, sg, 1e-5)
                p3 = spool.tile([PC, NCH * H], F32, tag="p3")
                nc.scalar.activation(p3, sg2, AF.Exp, scale=LN3)
                sm1 = spool.tile([PC, NCH * H], F32, tag="sm1")
                nc.vector.tensor_scalar_add(sm1, p3, -1.0)
                s2 = spool.tile([PC, NCH * H], F32, tag="s2")
                nc.vector.tensor_tensor(s2, sm1, sm1, OP.mult)
                r2 = spool.tile([PC, NCH * H], F32, tag="r2")
                nc.vector.reciprocal(r2, s2)
                mm = spool.tile([PC, NCH * H], F32, tag="mm")
                nc.vector.tensor_scalar_mul(mm, r2, -0.5)
            oslab = []
            for li in range(NCH):
                ot = outp.tile([PC, H * E], F32, tag=f"o{li}")
                oslab.append(ot)

            for h in range(H):
                th = h // 2
                po = 64 * (h % 2)
                vc = EXT * h

                # ---- E path in [s, l]: S^T = K Q^T, exp -> lhsT tiles ----
                ETs = []
                for sj in range(NCH):
                    nw = L - sj * PC
                    S = ps_s.tile([PC, L], F32, tag="S")
                    lh = KT[th][po:po + 64, sj * PC:(sj + 1) * PC]
                    rh = QT[th][po:po + 64, sj * PC:L]
                    nc.tensor.matmul(S[:, 0:PC], lh, rh[:, 0:PC],
                                     start=True, stop=False)
                    nc.tensor.matmul(S[:, 0:PC], ident, mtri, start=False, stop=True)
                    if nw > PC:
                        nc.tensor.matmul(S[:, PC:nw], lh, rh[:, PC:nw],
                                         start=True, stop=True)
                    et = etpool.tile([PC, L], BF16, tag=f"ET{sj}")
                    nc.scalar.activation(et[:, 0:nw], S[:, 0:nw], AF.Exp, scale=0.125)
                    ETs.append(et)

                # ---- G path in [l, s]: premul + one exp + transpose ----
                gtmp = gpool.tile([PC, NCH * BAND], F32, tag="gtmp")
                for li in range(NCH):
                    var = 0 if li == 0 else (2 if li == 3 else 1)
                    nc.gpsimd.tensor_scalar_mul(
                        gtmp[:, li * BAND:(li + 1) * BAND],
                        d2sb[:, var * BAND:(var + 1) * BAND],
                        mm[:, li * H + h:li * H + h + 1],
                    )
                g4 = gpool.tile([PC, NCH * BAND], BF16, tag="g4")
                nc.scalar.activation(g4, gtmp, AF.Exp)
                gt_ps = ps_t.tile([PC, 1024], BF16, tag="Tps")
                for li in range(NCH):
                    nc.tensor.transpose(
                        gt_ps[:, li * PC:(li + 1) * PC],
                        g4[:, li * BAND:li * BAND + PC], ident,
                    )
                    nc.tensor.transpose(
                        gt_ps[0:32, L + li * PC:L + (li + 1) * PC],
                        g4[:, li * BAND + PC:(li + 1) * BAND], ident,
                    )
                gts = gtpool.tile([PC, 1024], BF16, tag="gts")
                nc.vector.tensor_copy(gts, gt_ps)

                # ---- PV matmuls with ones column: U_ext [128, 65] per chunk ----
                U1 = ps_u.tile([PC, L], F32, tag="U1")
                U2 = ps_u.tile([PC, L], F32, tag="U2")
                def et_slice(sj, li):
                    return ETs[sj][:, (li - sj) * PC:(li - sj + 1) * PC]

                for li in range(NCH):
                    for sj in range(li + 1):
                        nc.tensor.matmul(
                            U1[:, li * PC:li * PC + EXT],
                            et_slice(sj, li),
                            Vn[sj][:, vc:vc + EXT],
                            start=(sj == 0), stop=(sj == li),
                        )
                    nc.tensor.matmul(
                        U2[:, li * PC:li * PC + EXT],
                        gts[:, li * PC:(li + 1) * PC],
                        Vs[li][:, vc:vc + EXT],
                        start=True, stop=False,
                    )
                    nc.tensor.matmul(
                        U2[:, li * PC:li * PC + EXT],
                        gts[0:32, L + li * PC:L + (li + 1) * PC],
                        Vs[li + 1][0:32, vc:vc + EXT],
                        start=False, stop=True,
                    )

                # ---- normalization scalars from ones-column sums ----
                re = small.tile([PC, NCH], F32, tag="re")
                nc.vector.reciprocal(
                    re, U1[:, :].rearrange("p (c w) -> p c w", w=PC)[:, :, E]
                )
                rg = small.tile([PC, NCH], F32, tag="rg")
                nc.vector.reciprocal(
                    rg, U2[:, :].rearrange("p (c w) -> p c w", w=PC)[:, :, E]
                )
                av = small.tile([PC, NCH], F32, tag="av")
                nc.vector.tensor_scalar_mul(av, re, gates_b[:, h:h + 1])
                bv = small.tile([PC, NCH], F32, tag="bv")
                nc.vector.tensor_scalar_mul(bv, rg, omg_b[:, h:h + 1])

                for li in range(NCH):
                    t2 = tmpp.tile([PC, E], F32, tag="t2")
                    if bi == 1 and h >= 6:
                        nc.scalar.activation(
                            t2, U2[:, li * PC:li * PC + E], AF.Copy,
                            bias=0.0, scale=bv[:, li:li + 1],
                        )
                    else:
                        nc.vector.tensor_scalar_mul(
                            t2, U2[:, li * PC:li * PC + E], bv[:, li:li + 1]
                        )
                    nc.vector.scalar_tensor_tensor(
                        oslab[li][:, h * E:(h + 1) * E],
                        U1[:, li * PC:li * PC + E],
                        av[:, li:li + 1], t2, OP.mult, OP.add,
                    )

                if h == 3:
                    for li in range(NCH):
                        nc.sync.dma_start(
                            out_h[bi, li * PC:(li + 1) * PC, 0:4, :],
                            oslab[li][:, 0:4 * E],
                        )
            for li in range(NCH):
                nc.sync.dma_start(
                    out_h[bi, li * PC:(li + 1) * PC, 4:8, :], oslab[li][:, 4 * E:]
                )

    nc.compile()
    _CACHE["nc"] = nc
    return nc


def kernel(**inputs):
    global LAST_RESULT
    nc = _build()
    q = np.ascontiguousarray(inputs["queries"], dtype=np.float32)
    k = np.ascontiguousarray(inputs["keys"], dtype=np.float32)
    v = np.ascontiguousarray(inputs["values"], dtype=np.float32)
    sg = np.ascontiguousarray(inputs["sigma"], dtype=np.float32)
    hgl = np.ascontiguousarray(inputs["head_gate_logit"], dtype=np.float32).reshape(1, H)

    in_maps = []
    for c in range(NCORES):
        b0 = BPC * c
        in_maps.append({
            "queries": q[b0:b0 + BPC],
            "keys": k[b0:b0 + BPC],
            "values": v[b0:b0 + BPC],
            "sigma": sg[b0:b0 + BPC],
            "hgl": hgl,
        })
    res = run_bass_kernel_spmd(nc, in_maps, core_ids=list(range(NCORES)))
    LAST_RESULT = res
    out = np.concatenate([r["out"] for r in res.results], axis=0)
    return out.astype(np.float32)


# revision 33
# speedup vs baseline: 24540.8117x; 23413.6536x over previous
"""AnomalyAttention Trainium2 kernel (8 NeuronCores, SPMD data-parallel over batch).

Math (per b,h):
  series = softmax(causal_mask(Q K^T / 8))          = E / sum(E)
  prior  = rownorm(exp(-(l-s)^2 / (2 sigma'^2)))    = G / sum(G)   (banded: |l-s|<=16 matters)
  fused  = g*series + (1-g)*prior ; renormalize     (sum == 1 -> renorm skipped, err ~1e-6)
  out    = fused @ V = a*(E@V) + b*(G@V),  a = g/sum(E), b = (1-g)/sum(G)  per row.

v3 structure:
  - scores computed TRANSPOSED (S^T = K Q^T, [s,l] layout) so exp(S^T) in SBUF is
    directly the lhsT of the PV matmul: no PE transposes / PSUM round trip for E.
  - row sums obtained via a ones-column appended to V (U_ext = A^T @ [V|1]):
    sum lands in column 64 of the PSUM result; no ACT accumulator reads.
  - Gaussian prior in [l,s] band layout (160 window per 128-chunk), premultiplied
    input (m * d2 on gpsimd), ONE merged exp per pair, PE-transposed to [s,l].
  - bf16 matmuls; normalization applied after PV on [128,64] tiles.
"""

import math
from contextlib import ExitStack

import ml_dtypes
import numpy as np

import concourse.bass as bass
import concourse.mybir as mybir
import concourse.tile as tile
from concourse import bacc
from concourse.bass_utils import run_bass_kernel_spmd

F32 = mybir.dt.float32
BF16 = mybir.dt.bfloat16
AF = mybir.ActivationFunctionType
OP = mybir.AluOpType

B, L, H, E = 16, 512, 8, 64
NCORES = 8
BPC = B // NCORES  # batches per core
PC = 128           # partition chunk
NCH = L // PC      # 4 chunks of 128 rows
BAND = 160         # gaussian band window (s in [128*li-16, 128*li+144))
BOFF = 16
EXT = 65           # V columns + ones column
MASKVAL = -240.0   # exp(0.125*(x-240)) <= e^-24 ~ 0
LN3 = math.log(3.0)

_CACHE = {}
LAST_RESULT = None


def _consts():
    ident = np.eye(PC, dtype=ml_dtypes.bfloat16)
    # mask for S^T diag block: -240 where l < s  (strict lower triangle: col j < row i)
    mtri_t = np.tril(np.full((PC, PC), MASKVAL, dtype=np.float32), k=-1).astype(
        ml_dtypes.bfloat16
    )
    # dist2 variants [3, 128, BAND]: d2[p, j] = (j - 16 - p)^2 ; poisoned out-of-range
    p = np.arange(PC)[:, None]
    j = np.arange(BAND)[None, :]
    d2 = ((j - BOFF - p) ** 2).astype(np.float32)
    d2_first = d2.copy()
    d2_first[:, :BOFF] = 1e30  # li=0: s = j-16 < 0 invalid
    d2_last = d2.copy()
    d2_last[:, 144:] = 1e30    # li=3: s = 352+j >= 512 invalid (j >= 144+16)
    dist2 = np.stack([d2_first, d2, d2_last])
    ones = np.ones((1, PC), dtype=np.float32)
    return ident, mtri_t, dist2, ones


def _build():
    if "nc" in _CACHE:
        return _CACHE["nc"]
    nc = bacc.Bacc()
    ident_np, mtri_np, dist2_np, ones_np = _consts()

    q_h = nc.dram_tensor("queries", [BPC, L, H, E], F32, kind="ExternalInput")
    k_h = nc.dram_tensor("keys", [BPC, L, H, E], F32, kind="ExternalInput")
    v_h = nc.dram_tensor("values", [BPC, L, H, E], F32, kind="ExternalInput")
    sig_h = nc.dram_tensor("sigma", [BPC, L, H], F32, kind="ExternalInput")
    hgl_h = nc.dram_tensor("hgl", [1, H], F32, kind="ExternalInput")
    out_h = nc.dram_tensor("out", [BPC, L, H, E], F32, kind="ExternalOutput")

    ident_d = nc.inline_tensor(ident_np, name="identc")
    mtri_d = nc.inline_tensor(mtri_np, name="mtric")
    dist2_d = nc.inline_tensor(dist2_np, name="dist2c")
    ones_d = nc.inline_tensor(ones_np, name="onesc")

    with ExitStack() as ctx:
        tc = ctx.enter_context(tile.TileContext(nc))
        const = ctx.enter_context(tc.tile_pool(name="const", bufs=1))
        qkT = ctx.enter_context(tc.tile_pool(name="qkT", bufs=2))
        vpool = ctx.enter_context(tc.tile_pool(name="vpool", bufs=2))
        spool = ctx.enter_context(tc.tile_pool(name="spool", bufs=2))
        etpool = ctx.enter_context(tc.tile_pool(name="etpool", bufs=4))
        gpool = ctx.enter_context(tc.tile_pool(name="gpool", bufs=4))
        gtpool = ctx.enter_context(tc.tile_pool(name="gtpool", bufs=3))
        small = ctx.enter_context(tc.tile_pool(name="small", bufs=4))
        outp = ctx.enter_context(tc.tile_pool(name="outp", bufs=2))
        tmpp = ctx.enter_context(tc.tile_pool(name="tmpp", bufs=5))
        ps_s = ctx.enter_context(tc.tile_pool(name="ps_s", bufs=2, space="PSUM"))
        ps_t = ctx.enter_context(tc.tile_pool(name="ps_t", bufs=2, space="PSUM"))
        ps_u = ctx.enter_context(tc.tile_pool(name="ps_u", bufs=2, space="PSUM"))
        dram = ctx.enter_context(tc.tile_pool(name="dram", bufs=2, space="DRAM"))

        # ---- sigma -> m for both batches (hoisted: fast DMA, tiny ops) ----
        m_of = {}
        for bi in range(BPC):
                sraw = spool.tile([PC, NCH * H], F32, tag="sraw")
                for c in range(NCH):
                    nc.sync.dma_start(
                        sraw[:, c * H:(c + 1) * H], sig_h[bi, c * PC:(c + 1) * PC, :]
                    )
                e5 = spool.tile([PC, NCH * H], F32, tag="e5")
                nc.scalar.activation(e5, sraw, AF.Exp, scale=-5.0)
                p1 = spool.tile([PC, NCH * H], F32, tag="p1")
                nc.vector.tensor_scalar_add(p1, e5, 1.0)
                sg = spool.tile([PC, NCH * H], F32, tag="sg")
                nc.vector.reciprocal(sg, p1)
                sg2 = spool.tile([PC, NCH * H], F32, tag="sg2")
                nc.vector.tensor_scalar_add(sg2, sg, 1e-5)
                p3 = spool.tile([PC, NCH * H], F32, tag="p3")
                nc.scalar.activation(p3, sg2, AF.Exp, scale=LN3)
                sm1 = spool.tile([PC, NCH * H], F32, tag="sm1")
                nc.vector.tensor_scalar_add(sm1, p3, -1.0)
                s2 = spool.tile([PC, NCH * H], F32, tag="s2")
                nc.vector.tensor_tensor(s2, sm1, sm1, OP.mult)
                r2 = spool.tile([PC, NCH * H], F32, tag="r2")
                nc.vector.reciprocal(r2, s2)
                mm = spool.tile([PC, NCH * H], F32, tag="mm")
                nc.vector.tensor_scalar_mul(mm, r2, -0.5)

                m_of[bi] = mm

        # ---- constants ----
        ident = const.tile([PC, PC], BF16, tag="ident")
        nc.scalar.dma_start(ident, ident_d[:, :])
        mtri = const.tile([PC, PC], BF16, tag="mtri")
        nc.scalar.dma_start(mtri, mtri_d[:, :])
        d2sb = const.tile([PC, 3 * BAND], F32, tag="d2sb")
        for v in range(3):
            nc.scalar.dma_start(d2sb[:, v * BAND:(v + 1) * BAND], dist2_d[v, :, :])
        ones_sb = const.tile([1, PC], F32, tag="ones")
        nc.scalar.dma_start(ones_sb, ones_d[:, :])

        # ---- gates ----
        hgl_sb = const.tile([1, H], F32, tag="hgl")
        nc.scalar.dma_start(hgl_sb, hgl_h[:, :])
        ge = const.tile([1, H], F32, tag="ge")
        nc.scalar.activation(ge, hgl_sb, AF.Exp, scale=-1.0)
        gp = const.tile([1, H], F32, tag="gp")
        nc.vector.tensor_scalar_add(gp, ge, 1.0)
        gate = const.tile([1, H], F32, tag="gate")
        nc.vector.reciprocal(gate, gp)  # sigmoid
        gb_ps = ps_s.tile([PC, L], F32, tag="S")
        nc.tensor.matmul(gb_ps[:, 0:H], ones_sb, gate, start=True, stop=True)
        gates_b = const.tile([PC, H], F32, tag="gatesb")
        nc.vector.tensor_copy(gates_b, gb_ps[:, 0:H])
        omg_b = const.tile([PC, H], F32, tag="omgb")
        nc.vector.tensor_scalar(omg_b, gates_b, -1.0, 1.0, OP.mult, OP.add)

        for bi in range(BPC):
            # ---- Q/K: SWDGE cast f32->bf16 straight to DRAM scratch ----
            qscr = dram.tile([L, H * E], BF16, tag="qscr")
            kscr = dram.tile([L, H * E], BF16, tag="kscr")
            for to in range(4):
                nc.gpsimd.dma_start(
                    qscr[:, to * PC:(to + 1) * PC],
                    q_h[bi, :, 2 * to:2 * to + 2, :],
                )
                nc.gpsimd.dma_start(
                    kscr[:, to * PC:(to + 1) * PC],
                    k_h[bi, :, 2 * to:2 * to + 2, :],
                )
            QT = []  # four [128, 512] tiles, one per head pair
            KT = []
            for to in range(4):
                qt = qkT.tile([PC, L], BF16, tag=f"qT{to}")
                kt = qkT.tile([PC, L], BF16, tag=f"kT{to}")
                eng = nc.scalar if bi == 0 else nc.sync
                eng.dma_start_transpose(qt, qscr[:, to * PC:(to + 1) * PC])
                eng.dma_start_transpose(kt, kscr[:, to * PC:(to + 1) * PC])
                QT.append(qt)
                KT.append(kt)

            # ---- V with ones column appended per head: [128, 8*65] ----
            # Vn_ext: natural rows; Vs_ext: rows shifted by -16 (5 tiles)
            Vn = []
            for t in range(4):
                vn = vpool.tile([PC, H * EXT], BF16, tag=f"vn{t}")
                nc.gpsimd.dma_start(
                    vn[:, :].rearrange("p (h e) -> p h e", h=H)[:, :, 0:E],
                    v_h[bi, t * PC:(t + 1) * PC, :, :],
                )
                nc.gpsimd.memset(
                    vn[:, :].rearrange("p (h e) -> p h e", h=H)[:, :, E:EXT], 1.0
                )
                Vn.append(vn)
            Vs = []
            for t in range(5):
                vs = vpool.tile([PC, H * EXT], BF16, tag=f"vs{t}")
                vs3 = vs[:, :].rearrange("p (h e) -> p h e", h=H)
                if t in (0, 4):
                    # edge zero-pad rows first, then ones columns on top
                    nc.gpsimd.memset(vs[0:32, :], 0.0)
                nc.gpsimd.memset(vs3[:, :, E:EXT], 1.0)
                if t == 0:
                    nc.gpsimd.dma_start(
                        vs3[BOFF:PC, :, 0:E], v_h[bi, 0:PC - BOFF, :, :]
                    )
                elif t == 4:
                    nc.gpsimd.dma_start(
                        vs3[0:BOFF, :, 0:E], v_h[bi, L - BOFF:L, :, :]
                    )
                else:
                    nc.gpsimd.dma_start(
                        vs3[:, :, 0:E],
                        v_h[bi, t * PC - BOFF:(t + 1) * PC - BOFF, :, :],
                    )
                Vs.append(vs)

            if bi in m_of:
                mm = m_of[bi]
            else:
                sraw = spool.tile([PC, NCH * H], F32, tag="sraw")
                for c in range(NCH):
                    nc.sync.dma_start(
                        sraw[:, c * H:(c + 1) * H], sig_h[bi, c * PC:(c + 1) * PC, :]
                    )
                e5 = spool.tile([PC, NCH * H], F32, tag="e5")
                nc.scalar.activation(e5, sraw, AF.Exp, scale=-5.0)
                p1 = spool.tile([PC, NCH * H], F32, tag="p1")
                nc.vector.tensor_scalar_add(p1, e5, 1.0)
                sg = spool.tile([PC, NCH * H], F32, tag="sg")
                nc.vector.reciprocal(sg, p1)
                sg2 = spool.tile([PC, NCH * H], F32, tag="sg2")
                nc.vector.tensor_scalar_add(sg2, sg, 1e-5)
                p3 = spool.tile([PC, NCH * H], F32, tag="p3")
                nc.scalar.activation(p3, sg2, AF.Exp, scale=LN3)
                sm1 = spool.tile([PC, NCH * H], F32, tag="sm1")
                nc.vector.tensor_scalar_add(sm1, p3, -1.0)
                s2 = spool.tile([PC, NCH * H], F32, tag="s2")
                nc.vector.tensor_tensor(s2, sm1, sm1, OP.mult)
                r2 = spool.tile([PC, NCH * H], F32, tag="r2")
                nc.vector.reciprocal(r2, s2)
                mm = spool.tile([PC, NCH * H], F32, tag="mm")
                nc.vector.tensor_scalar_mul(mm, r2, -0.5)
            oslab = []
            for li in range(NCH):
                ot = outp.tile([PC, H * E], F32, tag=f"o{li}")
                oslab.append(ot)

            for h in range(H):
                th = h // 2
                po = 64 * (h % 2)
                vc = EXT * h

                # ---- E path in [s, l]: S^T = K Q^T, exp -> lhsT tiles ----
                ETs = []
                for sj in range(2):
                    nw = L - sj * PC
                    S = ps_s.tile([PC, L], F32, tag="S")
                    lh = KT[th][po:po + 64, sj * PC:(sj + 1) * PC]
                    rh = QT[th][po:po + 64, sj * PC:L]
                    nc.tensor.matmul(S[:, 0:PC], lh, rh[:, 0:PC],
                                     start=True, stop=False)
                    nc.tensor.matmul(S[:, 0:PC], ident, mtri, start=False, stop=True)
                    nc.tensor.matmul(S[:, PC:nw], lh, rh[:, PC:nw],
                                     start=True, stop=True)
                    et = etpool.tile([PC, L], BF16, tag=f"ET{sj}")
                    nc.scalar.activation(et[:, 0:nw], S[:, 0:nw], AF.Exp, scale=0.125)
                    ETs.append((et, 0))
                # sj=2 ([0,256)) and sj=3 ([256,384)) share one bank + one exp
                S23 = ps_s.tile([PC, L], F32, tag="S")
                for sj in (2, 3):
                    off = (sj - 2) * 256
                    nw = L - sj * PC
                    lh = KT[th][po:po + 64, sj * PC:(sj + 1) * PC]
                    rh = QT[th][po:po + 64, sj * PC:L]
                    nc.tensor.matmul(S23[:, off:off + PC], lh, rh[:, 0:PC],
                                     start=True, stop=False)
                    nc.tensor.matmul(S23[:, off:off + PC], ident, mtri,
                                     start=False, stop=True)
                    if nw > PC:
                        nc.tensor.matmul(S23[:, off + PC:off + nw], lh, rh[:, PC:nw],
                                         start=True, stop=True)
                et23 = etpool.tile([PC, 384], BF16, tag="ET23")
                nc.scalar.activation(et23, S23[:, 0:384], AF.Exp, scale=0.125)
                ETs.append((et23, 0))
                ETs.append((et23, 256))

                # ---- G path in [l, s]: premul + one exp + transpose ----
                gtmp = gpool.tile([PC, NCH * BAND], F32, tag="gtmp")
                for li in range(NCH):
                    var = 0 if li == 0 else (2 if li == 3 else 1)
                    nc.gpsimd.tensor_scalar_mul(
                        gtmp[:, li * BAND:(li + 1) * BAND],
                        d2sb[:, var * BAND:(var + 1) * BAND],
                        mm[:, li * H + h:li * H + h + 1],
                    )
                g4 = gpool.tile([PC, NCH * BAND], BF16, tag="g4")
                nc.scalar.activation(g4, gtmp, AF.Exp)
                gt_ps = ps_t.tile([PC, 1024], BF16, tag="Tps")
                for li in range(NCH):
                    nc.tensor.transpose(
                        gt_ps[:, li * PC:(li + 1) * PC],
                        g4[:, li * BAND:li * BAND + PC], ident,
                    )
                    nc.tensor.transpose(
                        gt_ps[0:32, L + li * PC:L + (li + 1) * PC],
                        g4[:, li * BAND + PC:(li + 1) * BAND], ident,
                    )
                gts = gtpool.tile([PC, 1024], BF16, tag="gts")
                nc.vector.tensor_copy(gts, gt_ps)

                # ---- PV matmuls with ones column: U_ext [128, 65] per chunk ----
                U1 = ps_u.tile([PC, L], F32, tag="U1")
                U2 = ps_u.tile([PC, L], F32, tag="U2")
                def et_slice(sj, li):
                    t, off = ETs[sj]
                    return t[:, off + (li - sj) * PC:off + (li - sj + 1) * PC]

                for li in range(NCH):
                    for sj in range(li + 1):
                        nc.tensor.matmul(
                            U1[:, li * PC:li * PC + EXT],
                            et_slice(sj, li),
                            Vn[sj][:, vc:vc + EXT],
                            start=(sj == 0), stop=(sj == li),
                        )
                    nc.tensor.matmul(
                        U2[:, li * PC:li * PC + EXT],
                        gts[:, li * PC:(li + 1) * PC],
                        Vs[li][:, vc:vc + EXT],
                        start=True, stop=False,
                    )
                    nc.tensor.matmul(
                        U2[:, li * PC:li * PC + EXT],
                        gts[0:32, L + li * PC:L + (li + 1) * PC],
                        Vs[li + 1][0:32, vc:vc + EXT],
                        start=False, stop=True,
                    )

                # ---- normalization scalars from ones-column sums ----
                re = small.tile([PC, NCH], F32, tag="re")
                nc.vector.reciprocal(
                    re, U1[:, :].rearrange("p (c w) -> p c w", w=PC)[:, :, E]
                )
                rg = small.tile([PC, NCH], F32, tag="rg")
                nc.vector.reciprocal(
                    rg, U2[:, :].rearrange("p (c w) -> p c w", w=PC)[:, :, E]
                )
                av = small.tile([PC, NCH], F32, tag="av")
                nc.vector.tensor_scalar_mul(av, re, gates_b[:, h:h + 1])
                bv = small.tile([PC, NCH], F32, tag="bv")
                nc.vector.tensor_scalar_mul(bv, rg, omg_b[:, h:h + 1])

                for li in range(NCH):
                    t2 = tmpp.tile([PC, E], F32, tag="t2")
                    if bi == 1 and h >= 6:
                        nc.scalar.activation(
                            t2, U2[:, li * PC:li * PC + E], AF.Copy,
                            bias=0.0, scale=bv[:, li:li + 1],
                        )
                    else:
                        nc.vector.tensor_scalar_mul(
                            t2, U2[:, li * PC:li * PC + E], bv[:, li:li + 1]
                        )
                    nc.vector.scalar_tensor_tensor(
                        oslab[li][:, h * E:(h + 1) * E],
                        U1[:, li * PC:li * PC + E],
                        av[:, li:li + 1], t2, OP.mult, OP.add,
                    )

                if h == 3:
                    for li in range(NCH):
                        nc.sync.dma_start(
                            out_h[bi, li * PC:(li + 1) * PC, 0:4, :],
                            oslab[li][:, 0:4 * E],
                        )
            for li in range(NCH):
                nc.sync.dma_start(
                    out_h[bi, li * PC:(li + 1) * PC, 4:8, :], oslab[li][:, 4 * E:]
                )

    nc.compile()
    _CACHE["nc"] = nc
    return nc


def kernel(**inputs):
    global LAST_RESULT
    nc = _build()
    q = np.ascontiguousarray(inputs["queries"], dtype=np.float32)
    k = np.ascontiguousarray(inputs["keys"], dtype=np.float32)
    v = np.ascontiguousarray(inputs["values"], dtype=np.float32)
    sg = np.ascontiguousarray(inputs["sigma"], dtype=np.float32)
    hgl = np.ascontiguousarray(inputs["head_gate_logit"], dtype=np.float32).reshape(1, H)

    in_maps = []
    for c in range(NCORES):
        b0 = BPC * c
        in_maps.append({
            "queries": q[b0:b0 + BPC],
            "keys": k[b0:b0 + BPC],
            "values": v[b0:b0 + BPC],
            "sigma": sg[b0:b0 + BPC],
            "hgl": hgl,
        })
    res = run_bass_kernel_spmd(nc, in_maps, core_ids=list(range(NCORES)))
    LAST_RESULT = res
    out = np.concatenate([r["out"] for r in res.results], axis=0)
    return out.astype(np.float32)
